# Initial kernel scaffold
#
"""Trainium2 Bass kernel for nn_DipolePredictorSE3 (SE(3)-invariant sparse
graph attention + pooled MLP head).

Contract: kernel(**inputs) takes FULL unsharded inputs (B=16 graphs) and
returns the FULL [16, 3] float32 output. Internally shards 2 graphs per
NeuronCore across 8 cores (data parallel over batch), runs one SPMD Bass
program via run_bass_kernel_spmd, and finishes the tiny pooled MLP head on
host.

Math notes (vs reference):
  - neigh = adj | (adj@adj > 0) | eye  ==  ((adj|I) @ (adj|I)) > 0  exactly.
    M = adj|I is 0/1, exact in fp8; P = M@M accumulated in f32 PSUM is an
    exact integer count, so the mask min(P,1) is exact.
  - b_r2 (and any constant shift) cancels in softmax -> dropped.
  - With b_r1 == 0 (the setup default), rbias(d) = c*d with
    c = sum_{r: w1r>0} w1r*w2r, since d = dist > 0. A general fallback path
    evaluates the 8-term radial MLP when b_r1 != 0.
  - Softmax is computed unnormalized with a fixed logit shift (exact in the
    ratio); the denominator is obtained by a ones-column appended to V inside
    the aggregation matmul. Scores are produced transposed (S[j,i]) which
    makes the aggregation matmul transpose-free; this is valid because the
    mask and rbias are symmetric.
"""

import os
import sys

import numpy as np

for _p in ("/opt/trn_rl_repo", "/root/.axon_site/_ro/trn_rl_repo"):
    if os.path.isdir(_p) and _p not in sys.path:
        sys.path.insert(0, _p)

import concourse.bass as bass  # noqa: E402
import concourse.mybir as mybir  # noqa: E402
from concourse.bass_utils import run_bass_kernel_spmd  # noqa: E402
from concourse.tile import TileContext  # noqa: E402

B, N, D = 16, 1024, 5
NCORES = 8
GPC = B // NCORES  # graphs per core
AF = mybir.ActivationFunctionType
OP = mybir.AluOpType

SHIFT = 12.0  # constant logit shift folded into exp bias (exact in softmax)

last_results = None  # stashed BassKernelResults for test.py introspection


def _bf16_split(x):
    """Return (hi, lo) bf16 split of float32 array x (x ~= hi + lo)."""
    bf16 = mybir.dt.np(mybir.dt.bfloat16)
    hi = x.astype(bf16)
    lo = (x - hi.astype(np.float32)).astype(bf16)
    return hi, lo


def _build(c_val, fast_path, clamp, w1v, b1v, w2v):
    """Build the SPMD Bass program (per core: GPC graphs)."""
    nc = bass.Bass()
    f32 = mybir.dt.float32
    f32r = mybir.dt.float32r
    bf16 = mybir.dt.bfloat16
    fp8 = mybir.dt.float8e4
    fp16 = mybir.dt.float16

    KD = 20  # hi/lo split rows of the d2 operands

    m8 = nc.dram_tensor("m8", [GPC, N, N], fp8, kind="ExternalInput")
    ext = nc.dram_tensor("ext", [GPC, KD, 2 * N], bf16, kind="ExternalInput")
    fw = nc.dram_tensor("fw", [GPC, D, 3 * D + N], f32r, kind="ExternalInput")
    sdiag = nc.dram_tensor("sdiag", [128, 128], fp16, kind="ExternalInput")
    u_out = nc.dram_tensor("u_out", [GPC, D + 1, N], f32, kind="ExternalOutput")

    c2 = float(c_val) * float(c_val)
    sgn = 1.0 if c_val >= 0 else -1.0
    have_rb = (not fast_path) or (c_val != 0.0)

    def _split_matmul_waits():
        """Walrus's fused-matmul ISA struct holds only one sync wait; hoist
        extra waits onto preceding same-engine NoOps (identical sync
        semantics: engine queues execute in order)."""
        nid = [0]
        for blk in nc.m.functions[0].blocks:
            new_insts = []
            for ins in blk.instructions:
                si = ins.sync_info
                tn = type(ins).__name__
                splittable = tn not in (
                    "InstNoOp", "InstAllEngineBarrier",
                    "InstEventSemaphore", "InstTriggerDma",
                    "InstLoadActFuncSet",
                ) and getattr(ins, "engine", None) in (
                    mybir.EngineType.PE, mybir.EngineType.Activation,
                    mybir.EngineType.DVE, mybir.EngineType.Pool,
                    mybir.EngineType.SP,
                )
                if (
                    splittable
                    and si is not None
                    and si.on_wait
                    and len(si.on_wait) > 1
                ):
                    waits = list(si.on_wait)
                    for w in waits[:-1]:
                        nop = mybir.InstNoOp(
                            name=f"{ins.name}-wsplit{nid[0]}",
                            engine=ins.engine,
                            bass_nofuse=True,
                        )
                        nid[0] += 1
                        nop.sync_info = mybir.SyncInfo(on_wait=[w], on_update=[])
                        new_insts.append(nop)
                    ins.sync_info = mybir.SyncInfo(
                        on_wait=[waits[-1]], on_update=list(si.on_update)
                    )
                new_insts.append(ins)
            blk.instructions = new_insts

    with TileContext(nc) as tc:
        with (
            tc.tile_pool(name="pconst", bufs=1) as pconst,
            tc.tile_pool(name="psmall", bufs=2) as psmall,
            tc.tile_pool(name="pmt", bufs=2) as pmt,
            tc.tile_pool(name="pel", bufs=6) as pel,
            tc.tile_pool(name="ppd", bufs=2, space="PSUM") as ppd,
            tc.tile_pool(name="ppp", bufs=2, space="PSUM") as ppp,
            tc.tile_pool(name="ppc", bufs=2, space="PSUM") as ppc,
            tc.tile_pool(name="pagg", bufs=2, space="PSUM") as pagg,
        ):
            sdiag_t = pconst.tile([128, 128], fp16, name="sdiag_t")
            nc.sync.dma_start(sdiag_t, sdiag[:, :])

            def bias_tile(val, nm):
                t = pconst.tile([128, 1], f32, name=nm)
                nc.vector.memset(t, float(val))
                return t

            b_ln = bias_tile(1e-8, "b_ln")
            b_lnc = bias_tile(
                np.log(abs(c_val)) if (fast_path and c_val != 0.0) else 0.0,
                "b_lnc",
            )
            b_exp = bias_tile(-SHIFT, "b_exp")
            b_rad = [
                bias_tile(float(b1v_r), f"b_rad{r}")
                for r, b1v_r in enumerate(b1v)
            ] if not fast_path else []

            # ---- per-graph loads + q/k/v prep ----
            mts, exts, qts, kts, vexts = [], [], [], [], []
            for g in range(GPC):
                mt = pmt.tile([128, 8, N], fp8, tag="mt", name=f"mt{g}")
                nc.sync.dma_start(
                    mt, m8[g].rearrange("(s p) n -> p s n", p=128)
                )
                mts.append(mt)
                fw_t = psmall.tile([D, 3 * D + N], f32r, tag="fw", name=f"fw{g}")
                nc.sync.dma_start(fw_t, fw[g])
                wqkv_t = fw_t[:, 0 : 3 * D]
                ftt_t = fw_t[:, 3 * D :]
                if have_rb:
                    ext_t = psmall.tile([KD, 2 * N], bf16, tag="ext",
                                        name=f"ext{g}")
                    nc.sync.dma_start(ext_t, ext[g])
                    exts.append(ext_t)
                else:
                    exts.append(None)

                qt = psmall.tile([D, N], fp16, tag="qt", name=f"qt{g}")
                kt = psmall.tile([D, N], fp16, tag="kt", name=f"kt{g}")
                for ih in range(2):
                    sl = slice(ih * 512, (ih + 1) * 512)
                    pq = ppd.tile([D, 512], f32, tag="pd", name=f"pq{g}_{ih}")
                    nc.tensor.matmul(pq, wqkv_t[:, 0:D], ftt_t[:, sl],
                                     start=True, stop=True)
                    nc.scalar.copy(qt[:, sl], pq)
                    pk = ppd.tile([D, 512], f32, tag="pd", name=f"pk{g}_{ih}")
                    nc.tensor.matmul(pk, wqkv_t[:, D : 2 * D], ftt_t[:, sl],
                                     start=True, stop=True)
                    nc.scalar.copy(kt[:, sl], pk)
                qts.append(qt)
                kts.append(kt)

                vext = psmall.tile([128, 8, D + 1], bf16, tag="vext",
                                   name=f"vext{g}")
                nc.vector.memset(vext[:, :, D : D + 1], 1.0)
                pv = ppd.tile([128, 8, D], f32, tag="pd", name=f"pv{g}")
                for jc in range(8):
                    nc.tensor.matmul(
                        pv[:, jc, :],
                        ftt_t[:, jc * 128 : (jc + 1) * 128].bitcast(f32),
                        wqkv_t[:, 2 * D : 3 * D].bitcast(f32),
                        start=True, stop=True,
                    )
                nc.vector.tensor_copy(vext[:, :, 0:D], pv)
                vexts.append(vext)

            # ---- main software-pipelined tile loop ----
            tiles = [(g, jc, ih) for g in range(GPC) for jc in range(8)
                     for ih in range(2)]
            aggs = {}
            for g in range(GPC):
                for ih in range(2):
                    aggs[(g, ih)] = pagg.tile([D + 1, 512], f32, tag="agg",
                                              name=f"agg{g}_{ih}")
            pcs, ets, ems, rbs = {}, {}, {}, {}

            def emit_front(t):
                """D + P matmuls and their elementwise consumers for tile t."""
                g, jc, ih = tiles[t]
                jsl = slice(jc * 128, (jc + 1) * 128)
                isl = slice(ih * 512, (ih + 1) * 512)
                if have_rb:
                    pd2 = ppd.tile([128, 512], f32, tag="pd", name=f"pd_{t}")
                    nc.tensor.matmul(
                        pd2,
                        exts[g][:, 0:N][:, jsl],
                        exts[g][:, N : 2 * N][:, isl],
                        start=True, stop=True,
                    )
                pp = ppp.tile([128, 512], f32, tag="pp", name=f"pp_{t}")
                mt = mts[g]
                for s2 in range(8):
                    nc.tensor.matmul(
                        pp,
                        mt[:, s2, jsl],
                        mt[:, s2, isl],
                        start=(s2 == 0), stop=(s2 == 7),
                    )
                if have_rb:
                    if clamp:
                        d2c = pel.tile([128, 512], f32, tag="d2c",
                                       name=f"d2c_{t}")
                        nc.vector.tensor_single_scalar(d2c, pd2, 0.0, OP.max)
                        ln_in = d2c
                    else:
                        ln_in = pd2
                    lt = pel.tile([128, 512], f32, tag="lt", name=f"lt_{t}")
                    nc.scalar.activation(lt, ln_in, AF.Ln,
                                         bias=b_ln[: ln_in.shape[0], :])
                    # rb = |c| * dist = exp(0.5*ln(d2c+eps) + ln|c|)
                    rb = pel.tile([128, 512], fp16 if fast_path else f32,
                                  tag="rb", bufs=20, name=f"rb_{t}")
                    nc.scalar.activation(rb, lt, AF.Exp,
                                         bias=b_lnc[:128, :],
                                         scale=0.5)
                    rbs[t] = rb
                m01 = pel.tile([128, 512], bf16, tag="m01", bufs=20, name=f"m01_{t}")
                nc.vector.tensor_single_scalar(m01, pp, 1.0, OP.min)
                return m01

            m01s = {}

            def emit_content(t):
                g, jc, ih = tiles[t]
                jsl = slice(jc * 128, (jc + 1) * 128)
                isl = slice(ih * 512, (ih + 1) * 512)
                pc = ppc.tile([128, 512], f32, tag="pc", name=f"pc_{t}")
                rb_add = have_rb and fast_path
                nc.tensor.matmul(pc, kts[g][:, jsl], qts[g][:, isl],
                                 start=True, stop=not rb_add)
                pcs[t] = pc

            def emit_diag(t):
                if have_rb and fast_path:
                    nc.tensor.matmul(pcs[t], sdiag_t, rbs.pop(t),
                                     start=False, stop=True)

            def emit_scores(t):
                emit_content(t)
                emit_diag(t)

            def emit_exp(t):
                pc = pcs.pop(t)
                e_in = pc if fast_path else _fallback_logits(t, pc)
                et = pel.tile([128, 512], bf16, tag="et", name=f"et_{t}")
                nc.scalar.activation(et, e_in, AF.Exp,
                                     bias=b_exp[: e_in.shape[0], :])
                ets[t] = et

            def emit_mul(t):
                em = pel.tile([128, 512], bf16, tag="em", name=f"em_{t}")
                nc.vector.tensor_mul(em, ets.pop(t), m01s.pop(t))
                ems[t] = em

            def emit_exp_mask(t):
                emit_exp(t)
                emit_mul(t)

            def _fallback_logits(t, pc):
                # general radial MLP: dist = exp(0.5*L); sum_r w2r*relu(...)
                lt_prev = rbs.pop(t)  # holds f32 exp(0.5*ln) = dist
                racc = None
                for r in range(len(w1v)):
                    h = pel.tile([128, 512], f32, tag="hrad", bufs=4,
                                 name=f"h_{t}_{r}")
                    nc.scalar.activation(h, lt_prev, AF.Relu,
                                         bias=b_rad[r][:128, :],
                                         scale=float(w1v[r]))
                    nr = pel.tile([128, 512], f32, tag="racc", bufs=4,
                                  name=f"ra_{t}_{r}")
                    if racc is None:
                        nc.vector.tensor_single_scalar(
                            nr, h, float(w2v[r]), OP.mult)
                    else:
                        nc.vector.scalar_tensor_tensor(
                            nr, h, float(w2v[r]), racc, OP.mult, OP.add)
                    racc = nr
                s1 = pel.tile([128, 512], f32, tag="s1", name=f"s1_{t}")
                nc.vector.scalar_tensor_tensor(
                    s1, racc, 1.0, pc, OP.bypass, OP.add)
                return s1

            def emit_agg(t):
                g, jc, ih = tiles[t]
                nc.tensor.matmul(
                    aggs[(g, ih)],
                    vexts[g][:, jc, :],
                    ems.pop(t),
                    start=(jc == 0), stop=(jc == 7),
                )

            nT = len(tiles)
            if fast_path:
                # A(g0); then A(g1) interleaved with B(g0); then B(g1).
                # ln/exp share one ACT table set, so alternation is free.
                def emit_B(t):
                    emit_content(t)
                    emit_diag(t)
                    emit_exp(t)
                    emit_mul(t)
                    emit_agg(t)

                half = nT // 2
                for t in range(half):
                    m01s[t] = emit_front(t)
                for i in range(half):
                    m01s[half + i] = emit_front(half + i)
                    emit_B(i)
                for i in range(half, nT):
                    emit_B(i)
            else:
                for t in range(nT):
                    m01s[t] = emit_front(t)
                    emit_scores(t)
                    emit_exp_mask(t)
                    emit_agg(t)

            for g in range(GPC):
                for ih in range(2):
                    u_sb = pel.tile([D + 1, 512], f32, tag="usb", bufs=2,
                                    name=f"usb{g}_{ih}")
                    nc.vector.tensor_copy(u_sb, aggs[(g, ih)])
                    nc.sync.dma_start(
                        u_out[g, :, ih * 512 : (ih + 1) * 512], u_sb
                    )
    _split_matmul_waits()
    return nc


def kernel(
    feats, coors, adj_mat, Wq, Wk, Wv, Wo,
    w_r1, b_r1, w_r2, b_r2, w1, b1, w2, b2,
):
    global last_results
    f32 = np.float32
    fp8np = mybir.dt.np(mybir.dt.float8e4)

    feats = np.asarray(feats, dtype=f32)
    coors = np.asarray(coors, dtype=f32)
    adj = np.asarray(adj_mat).astype(bool)
    Wq = np.asarray(Wq, f32); Wk = np.asarray(Wk, f32)
    Wv = np.asarray(Wv, f32); Wo = np.asarray(Wo, f32)
    w_r1 = np.asarray(w_r1, f32); b_r1 = np.asarray(b_r1, f32)
    w_r2 = np.asarray(w_r2, f32); b_r2 = np.asarray(b_r2, f32)
    w1 = np.asarray(w1, f32); b1 = np.asarray(b1, f32)
    w2 = np.asarray(w2, f32); b2 = np.asarray(b2, f32)

    # ---- host layout prep (no model compute beyond O(B*N)) ----
    eye = np.eye(N, dtype=bool)
    m8 = (adj | eye[None]).astype(fp8np)  # [B,N,N] fp8 {0,1}

    fttT = np.ascontiguousarray(feats.transpose(0, 2, 1))  # [B,5,N]

    n2 = (coors * coors).sum(-1)  # [B,N]
    ones = np.ones_like(n2)
    ct = coors.transpose(0, 2, 1)  # [B,3,N]
    extL = np.concatenate([-2.0 * ct, n2[:, None], ones[:, None]], axis=1)
    extR = np.concatenate([ct, ones[:, None], n2[:, None]], axis=1)  # [B,5,N]
    lhi, llo = _bf16_split(extL)
    rhi, rlo = _bf16_split(extR)
    # sum over 20 rows = LhiRhi + LhiRlo + LloRhi + LloRlo
    extl20 = np.concatenate([lhi, lhi, llo, llo], axis=1)  # [B,20,N]
    extr20 = np.concatenate([rhi, rlo, rhi, rlo], axis=1)

    wqkv = np.concatenate([Wq / np.sqrt(D), Wk, Wv], axis=1).astype(f32)

    # radial MLP fast-path constant: rbias(d) = c*d (+const) when b_r1 == 0
    w1v = w_r1[0]  # [8]
    w2v = w_r2[:, 0]  # [8]
    fast_path = bool(np.all(b_r1 == 0.0))
    c_val = float(np.sum(np.where(w1v > 0, w1v * w2v, 0.0)))

    sgn = np.float32(1.0 if c_val >= 0 else -1.0)
    fp16np = mybir.dt.np(mybir.dt.float16)
    sdiag = (sgn * np.eye(128, dtype=np.float32)).astype(fp16np)

    nc = _build(c_val, fast_path, True, w1v, b_r1, w2v)

    ext40 = np.concatenate([extl20, extr20], axis=2)  # [B,20,2N]
    fw = np.concatenate(
        [np.broadcast_to(wqkv[None], (B, D, 3 * D)), fttT], axis=2
    ).astype(f32)  # [B,5,15+N]

    in_maps = []
    for core in range(NCORES):
        gs = slice(core * GPC, (core + 1) * GPC)
        in_maps.append(
            {
                "m8": np.ascontiguousarray(m8[gs]),
                "ext": np.ascontiguousarray(ext40[gs]),
                "fw": np.ascontiguousarray(fw[gs]),
                "sdiag": sdiag,
            }
        )

    trace = bool(os.environ.get("BASS_TRACE"))
    res = run_bass_kernel_spmd(nc, in_maps, list(range(NCORES)), trace=trace)
    last_results = res

    # ---- host finish: normalize, pool, tiny MLP head ----
    u = np.stack([r["u_out"] for r in res.results]).reshape(B, D + 1, N)
    aggT = u[:, 0:D, :] / u[:, D : D + 1, :]  # [B,5,N]
    agg_mean = aggT.mean(axis=2)  # [B,5]
    pooled = feats.mean(axis=1) + agg_mean @ Wo  # [B,5]
    hdn = np.maximum(pooled @ w1 + b1, 0.0)
    out = hdn @ w2 + b2
    return out.astype(f32)



# revision 13
# speedup vs baseline: 1.1908x; 1.1908x over previous
"""Trainium2 Bass kernel for nn_DipolePredictorSE3 (SE(3)-invariant sparse
graph attention + pooled MLP head).

Contract: kernel(**inputs) takes FULL unsharded inputs (B=16 graphs) and
returns the FULL [16, 3] float32 output. Internally shards 2 graphs per
NeuronCore across 8 cores (data parallel over batch), runs one SPMD Bass
program via run_bass_kernel_spmd, and finishes the tiny pooled MLP head on
host.

Math notes (vs reference):
  - neigh = adj | (adj@adj > 0) | eye  ==  ((adj|I) @ (adj|I)) > 0  exactly.
    M = adj|I is 0/1, exact in fp8; P = M@M accumulated in f32 PSUM is an
    exact integer count. The 2-hop mask matmul runs in fp8 DoubleRow perf
    mode (2 contraction chunks per instruction at 2x row rate).
  - b_r2 (and any constant shift) cancels in softmax -> dropped.
  - With b_r1 == 0 (the setup default), rbias(d) = c*d with
    c = sum_{r: w1r>0} w1r*w2r, since d = dist > 0. A general fallback path
    evaluates the 8-term radial MLP when b_r1 != 0.
  - Masking is folded into the logits additively: the DVE computes
    rbm = min(P,1) +/- rb/BIG in one scalar_tensor_tensor op, a BIG*I
    diagonal matmul adds BIG*rbm into the score PSUM, and the exp bias
    subtracts SHIFT+BIG. Masked-out entries get exp(s - BIG - SHIFT) ~ 0.
  - Softmax is computed unnormalized with a fixed logit shift (exact in the
    ratio); the denominator is obtained by a ones-column appended to V inside
    the aggregation matmul. Scores are produced transposed (S[j,i]) which
    makes the aggregation matmul transpose-free; this is valid because the
    mask and rbias are symmetric.
"""

import os
import sys

import numpy as np

for _p in ("/opt/trn_rl_repo", "/root/.axon_site/_ro/trn_rl_repo"):
    if os.path.isdir(_p) and _p not in sys.path:
        sys.path.insert(0, _p)

import concourse.bass as bass  # noqa: E402
import concourse.mybir as mybir  # noqa: E402
from concourse.bass_utils import run_bass_kernel_spmd  # noqa: E402
from concourse.tile import TileContext  # noqa: E402

B, N, D = 16, 1024, 5
NCORES = 8
GPC = B // NCORES  # graphs per core
AF = mybir.ActivationFunctionType
OP = mybir.AluOpType

SHIFT = 12.0  # constant logit shift folded into exp bias (exact in softmax)
BIG = 30.0   # additive mask suppression (exp(-BIG) ~ 1e-13)
EPS_D2 = 4e-3  # ln() input floor; also absorbs hi/lo d2 rounding (<~1e-3)

last_results = None  # stashed BassKernelResults for test.py introspection


def _bf16_split(x):
    """Return (hi, lo) bf16 split of float32 array x (x ~= hi + lo)."""
    bf16 = mybir.dt.np(mybir.dt.bfloat16)
    hi = x.astype(bf16)
    lo = (x - hi.astype(np.float32)).astype(bf16)
    return hi, lo


def _build(c_val, fast_path, w1v, b1v, w2v):
    """Build the SPMD Bass program (per core: GPC graphs)."""
    nc = bass.Bass()
    f32 = mybir.dt.float32
    f32r = mybir.dt.float32r
    bf16 = mybir.dt.bfloat16
    fp8 = mybir.dt.float8e4
    fp16 = mybir.dt.float16
    DR = mybir.MatmulPerfMode.DoubleRow

    KD = 20  # hi/lo split rows of the d2 operands

    m8 = nc.dram_tensor("m8", [GPC, N, N], fp8, kind="ExternalInput")
    ext = nc.dram_tensor("ext", [GPC, KD, 2 * N], bf16, kind="ExternalInput")
    fw = nc.dram_tensor("fw", [GPC, D, 3 * D + N], f32r, kind="ExternalInput")
    sdiag = nc.dram_tensor("sdiag", [128, 128], fp16, kind="ExternalInput")
    ident = nc.dram_tensor("ident", [128, 128], fp16, kind="ExternalInput")
    u_out = nc.dram_tensor("u_out", [GPC, D + 1, N], f32, kind="ExternalOutput")

    have_rb = (not fast_path) or (c_val != 0.0)

    def _split_matmul_waits():
        """Walrus's fused-matmul ISA struct holds only one sync wait; hoist
        extra waits onto preceding same-engine NoOps (identical sync
        semantics: engine queues execute in order)."""
        nid = [0]
        for blk in nc.m.functions[0].blocks:
            new_insts = []
            for ins in blk.instructions:
                si = ins.sync_info
                tn = type(ins).__name__
                splittable = tn not in (
                    "InstNoOp", "InstAllEngineBarrier",
                    "InstEventSemaphore", "InstTriggerDma",
                    "InstLoadActFuncSet",
                ) and getattr(ins, "engine", None) in (
                    mybir.EngineType.PE, mybir.EngineType.Activation,
                    mybir.EngineType.DVE, mybir.EngineType.Pool,
                    mybir.EngineType.SP,
                )
                if (
                    splittable
                    and si is not None
                    and si.on_wait
                    and len(si.on_wait) > 1
                ):
                    waits = list(si.on_wait)
                    for w in waits[:-1]:
                        nop = mybir.InstNoOp(
                            name=f"{ins.name}-wsplit{nid[0]}",
                            engine=ins.engine,
                            bass_nofuse=True,
                        )
                        nid[0] += 1
                        nop.sync_info = mybir.SyncInfo(on_wait=[w], on_update=[])
                        new_insts.append(nop)
                    ins.sync_info = mybir.SyncInfo(
                        on_wait=[waits[-1]], on_update=list(si.on_update)
                    )
                new_insts.append(ins)
            blk.instructions = new_insts
        return nc

    with TileContext(nc) as tc:
        with (
            tc.tile_pool(name="pconst", bufs=1) as pconst,
            tc.tile_pool(name="psmall", bufs=2) as psmall,
            tc.tile_pool(name="pmt", bufs=2) as pmt,
            tc.tile_pool(name="pel", bufs=6) as pel,
            tc.tile_pool(name="ppd", bufs=1, space="PSUM") as ppd,
            tc.tile_pool(name="ppp", bufs=2, space="PSUM") as ppp,
            tc.tile_pool(name="ppc", bufs=2, space="PSUM") as ppc,
            tc.tile_pool(name="pagg", bufs=2, space="PSUM") as pagg,
            tc.tile_pool(name="ptrp", bufs=1, space="PSUM") as ptr,
        ):
            sdiag_t = pconst.tile([128, 128], fp16, name="sdiag_t")
            nc.sync.dma_start(sdiag_t, sdiag[:, :])
            ident_t = pconst.tile([128, 128], fp16, name="ident_t")
            nc.sync.dma_start(ident_t, ident[:, :])

            def bias_tile(val, nm):
                t = pconst.tile([128, 1], f32, name=nm)
                nc.vector.memset(t, float(val))
                return t

            b_ln = bias_tile(EPS_D2 if fast_path else 1e-8, "b_ln")
            b_lnc = bias_tile(
                np.log(abs(c_val) / BIG) if (fast_path and c_val != 0.0)
                else 0.0,
                "b_lnc",
            )
            b_exp = bias_tile(-SHIFT, "b_exp")
            b_rad = [
                bias_tile(float(b1v_r), f"b_rad{r}")
                for r, b1v_r in enumerate(b1v)
            ] if not fast_path else []

            # ---- small per-graph loads first (fw, ext) so qkv prep can
            # start while the big adjacency DMAs stream in ----
            fws, exts = [], []
            for g in range(GPC):
                fw_t = psmall.tile([D, 3 * D + N], f32r, tag="fw", name=f"fw{g}")
                nc.sync.dma_start(fw_t, fw[g])
                fws.append(fw_t)
                if have_rb:
                    ext_t = psmall.tile([KD, 2 * N], bf16, tag="ext",
                                        name=f"ext{g}")
                    nc.sync.dma_start(ext_t, ext[g])
                    exts.append(ext_t)
                else:
                    exts.append(None)

            # ---- adjacency loads, split into column halves x k-chunk
            # pairs so the first P matmuls start as soon as ~128KB lands ----
            mts = []
            for g in range(GPC):
                halves = []
                srcg = m8[g].rearrange("(s p) n -> p s n", p=128)
                for hf in range(2):
                    chunks = []
                    for s2 in range(4):
                        mt = pmt.tile([128, 2, 512], fp8, tag="mt", bufs=16,
                                      name=f"mt{g}_{hf}_{s2}")
                        nc.scalar.dma_start(
                            mt,
                            srcg[:, 2 * s2 : 2 * s2 + 2,
                                 hf * 512 : (hf + 1) * 512],
                        )
                        chunks.append(mt)
                    halves.append(chunks)
                mts.append(halves)

            # ---- q/k/v prep ----
            content_fp8 = fast_path and bool(os.environ.get("CONTENT_FP8"))
            qts, kts, vexts = [], [], []
            for g in range(GPC):
                fw_t = fws[g]
                wqkv_t = fw_t[:, 0 : 3 * D]
                ftt_t = fw_t[:, 3 * D :]
                if content_fp8:
                    # fp8 DoubleRow layout: chunk0 = dims 0..2, chunk1 =
                    # dims 3,4 + a zero row
                    qt = psmall.tile([3, 2, N], fp8, tag="qt", name=f"qt{g}")
                    kt = psmall.tile([3, 2, N], fp8, tag="kt", name=f"kt{g}")
                    nc.vector.memset(qt[:, 1, :], 0.0)
                    nc.vector.memset(kt[:, 1, :], 0.0)
                else:
                    qt = psmall.tile([D, N], fp16, tag="qt", name=f"qt{g}")
                    kt = psmall.tile([D, N], fp16, tag="kt", name=f"kt{g}")
                for ih in range(2):
                    sl = slice(ih * 512, (ih + 1) * 512)
                    if content_fp8:
                        # split projections so each PSUM tile starts at
                        # partition 0 (engine partition-base restriction)
                        for (lo, hi, ch, wofs) in (
                            (0, 3, 0, 0), (3, 5, 1, 0),
                        ):
                            m = hi - lo
                            pq = ppd.tile([m, 512], f32, tag="pd",
                                          name=f"pq{g}_{ih}_{ch}")
                            nc.tensor.matmul(pq, wqkv_t[:, lo:hi],
                                             ftt_t[:, sl],
                                             start=True, stop=True)
                            nc.vector.tensor_copy(qt[0:m, ch, sl], pq)
                            pk = ppd.tile([m, 512], f32, tag="pd",
                                          name=f"pk{g}_{ih}_{ch}")
                            nc.tensor.matmul(pk, wqkv_t[:, D + lo : D + hi],
                                             ftt_t[:, sl],
                                             start=True, stop=True)
                            nc.vector.tensor_copy(kt[0:m, ch, sl], pk)
                    else:
                        pq = ppd.tile([D, 512], f32, tag="pd",
                                      name=f"pq{g}_{ih}")
                        nc.tensor.matmul(pq, wqkv_t[:, 0:D], ftt_t[:, sl],
                                         start=True, stop=True)
                        nc.vector.tensor_copy(qt[:, sl], pq)
                        pk = ppd.tile([D, 512], f32, tag="pd",
                                      name=f"pk{g}_{ih}")
                        nc.tensor.matmul(pk, wqkv_t[:, D : 2 * D],
                                         ftt_t[:, sl],
                                         start=True, stop=True)
                        nc.vector.tensor_copy(kt[:, sl], pk)
                qts.append(qt)
                kts.append(kt)

                vext = psmall.tile([128, 8, D + 1], bf16, tag="vext",
                                   name=f"vext{g}")
                nc.vector.memset(vext[:, :, D : D + 1], 1.0)
                pv = ppd.tile([128, 8, D], f32, tag="pd", name=f"pv{g}")
                for jc in range(8):
                    nc.tensor.matmul(
                        pv[:, jc, :],
                        ftt_t[:, jc * 128 : (jc + 1) * 128].bitcast(f32),
                        wqkv_t[:, 2 * D : 3 * D].bitcast(f32),
                        start=True, stop=True,
                    )
                nc.vector.tensor_copy(vext[:, :, 0:D], pv)
                vexts.append(vext)

            # ---- main software-pipelined tile loop ----
            # Exploit rbm symmetry: only the on/below-diagonal prefix of
            # each tile row is computed; upper blocks are transposed copies.
            # Order puts each mirror after its sources (descending partials).
            mirror = fast_path and not os.environ.get("NO_MIR")
            if mirror:
                order = (
                    [(3, 0), (2, 0), (1, 0), (0, 0)]
                    + [(jc, 0) for jc in range(4, 8)]
                    + [(7, 1), (6, 1), (5, 1), (4, 1)]
                    + [(jc, 1) for jc in range(4)]
                )
            else:
                order = (
                    [(jc, 0) for jc in range(4)]
                    + [(jc, 0) for jc in range(4, 8)]
                    + [(jc, 1) for jc in range(8)]
                )

            part = mirror and not os.environ.get("NO_PARTIAL")

            def cwidth(jc, ih):
                """Computed (non-mirrored) column prefix of tile (jc, ih)."""
                if not mirror:
                    return 512
                if ih == 0:
                    return min(jc + 1, 4) * 128 if part else 512
                if jc < 4:
                    return 0
                return (jc - 3) * 128 if part else 512

            # agg accumulation-group boundaries follow emission order
            ih_seq = {ih: [jc for (jc, i) in order if i == ih]
                      for ih in range(2)}
            tiles = [(g, jc, ih) for g in range(GPC) for (jc, ih) in order]
            aggs = {}
            for g in range(GPC):
                for ih in range(2):
                    aggs[(g, ih)] = pagg.tile([D + 1, 512], f32, tag="agg",
                                              name=f"agg{g}_{ih}")
            pcs, ets, ems, rbs_d, rbms = {}, {}, {}, {}, {}

            def tile_index(g, jc, ih):
                return g * 16 + order.index((jc, ih))

            def emit_mirror_blocks(t, rbm, b0):
                """Fill blocks [b0..3] of rbm via PE transposes of the
                symmetric source tiles (one PSUM staging tile + one copy)."""
                g, jc, ih = tiles[t]
                ptr_t = ptr.tile([128, 4, 128], fp16, tag="ptr",
                                 name=f"ptr_{t}")
                for b in range(b0, 4):
                    if ih == 0:
                        srct = rbms[tile_index(g, b, 0)]
                        ssl = slice(jc * 128, (jc + 1) * 128)
                    else:
                        srct = rbms[tile_index(g, 4 + b, 1)]
                        ssl = slice((jc - 4) * 128, (jc - 3) * 128)
                    nc.tensor.matmul(ptr_t[:, b, :], srct[:, ssl], ident_t,
                                     is_transpose=True)
                nc.vector.tensor_copy(
                    rbm[:, b0 * 128 : 512], ptr_t[:, b0:4, :])

            def emit_front(t):
                """P + d2 matmuls, Ln/Exp radial chain, fused mask+rb DVE.
                Only the computed prefix [0:W) is produced directly; the
                remaining blocks are PE/DMA transposes of symmetric
                sources."""
                g, jc, ih = tiles[t]
                W = cwidth(jc, ih)
                if W == 0:
                    # fully-upper tile: assemble rbm entirely from DMA
                    # transposes of the 4 source tiles (4+b, ih=0)
                    rbm = pel.tile([128, 512], fp16, tag="rbm", bufs=20,
                                   name=f"rbm_{t}")
                    for b in range(4):
                        srct = rbms[tile_index(g, 4 + b, 0)]
                        nc.sync.dma_start_transpose(
                            rbm[:, b * 128 : (b + 1) * 128],
                            srct[:, jc * 128 : (jc + 1) * 128],
                        )
                    rbms[t] = rbm
                    return
                jsl = slice(jc * 128, (jc + 1) * 128)
                isl = slice(ih * 512, ih * 512 + W)
                wsl = slice(0, W)
                if have_rb:
                    pd2 = ppd.tile([128, 512], f32, tag="pd", name=f"pd_{t}")
                    nc.tensor.matmul(
                        pd2[:, wsl],
                        exts[g][:, 0:N][:, jsl],
                        exts[g][:, N : 2 * N][:, isl],
                        start=True, stop=True,
                    )
                pp = ppp.tile([128, 512], f32, tag="pp", name=f"pp_{t}")
                lhsc = mts[g][jc // 4]
                rhsc = mts[g][ih]
                loff = (jc % 4) * 128
                for s2 in range(4):
                    nc.tensor.matmul(
                        pp[:, wsl],
                        lhsc[s2][:, :, loff : loff + 128],
                        rhsc[s2][:, :, wsl],
                        start=(s2 == 0), stop=(s2 == 3),
                        perf_mode=DR,
                    )
                if have_rb:
                    lt = pel.tile([128, 512], f32, tag="lt", name=f"lt_{t}")
                    nc.scalar.activation(lt[:, wsl], pd2[:, wsl], AF.Ln,
                                         bias=b_ln[:128, :])
                    if fast_path:
                        # rbs = |c| * sqrt(d2+eps) / BIG   (fp16: <= ~0.03)
                        rbs = pel.tile([128, 512], fp16, tag="rb", bufs=4,
                                       name=f"rb_{t}")
                        nc.scalar.activation(rbs[:, wsl], lt[:, wsl], AF.Exp,
                                             bias=b_lnc[:128, :],
                                             scale=0.5)
                        # t0 = min(P,1) - 1 in {0,-1}, exact in fp16
                        t0 = pel.tile([128, 512], fp16, tag="t0", bufs=4,
                                      name=f"t0_{t}")
                        nc.vector.tensor_scalar(
                            t0[:, wsl], pp[:, wsl], 1.0, 1.0,
                            OP.min, OP.subtract)
                        # rbm = t0 +/- rbs (sign of c); masked-in values are
                        # tiny so fp16 ulp noise is ~BIG*1.5e-5 logits
                        rbm = pel.tile([128, 512], fp16, tag="rbm", bufs=20,
                                       name=f"rbm_{t}")
                        nc.vector.tensor_tensor(
                            rbm[:, wsl], t0[:, wsl], rbs[:, wsl],
                            OP.add if c_val >= 0 else OP.subtract)
                        rbms[t] = rbm
                        if W < 512:
                            emit_mirror_blocks(t, rbm, W // 128)
                        return
                    # fallback keeps dist = exp(0.5*ln(d2c+1e-8))
                    rbf = pel.tile([128, 512], f32, tag="rb", bufs=4,
                                   name=f"rb_{t}")
                    nc.scalar.activation(rbf, lt, AF.Exp, scale=0.5)
                    rbs_d[t] = rbf
                if not fast_path:
                    m01 = pel.tile([128, 512], bf16, tag="m01", bufs=20,
                                   name=f"m01_{t}")
                    nc.vector.tensor_single_scalar(m01, pp, 1.0, OP.min)
                    rbms[t] = m01
                else:
                    # fast path without radial bias: rbm = min(P,1)-1 in fp16
                    rbm = pel.tile([128, 512], fp16, tag="rbm", bufs=20,
                                   name=f"rbm_{t}")
                    nc.vector.tensor_scalar(
                        rbm[:, wsl], pp[:, wsl], 1.0, 1.0,
                        OP.min, OP.subtract)
                    rbms[t] = rbm
                    if W < 512:
                        emit_mirror_blocks(t, rbm, W // 128)

            def emit_back(t):
                g, jc, ih = tiles[t]
                jsl = slice(jc * 128, (jc + 1) * 128)
                isl = slice(ih * 512, (ih + 1) * 512)
                pc = ppc.tile([128, 512], f32, tag="pc", name=f"pc_{t}")
                if fast_path:
                    if content_fp8:
                        nc.tensor.matmul(pc, kts[g][:, :, jsl],
                                         qts[g][:, :, isl],
                                         start=True, stop=False,
                                         perf_mode=DR)
                    else:
                        nc.tensor.matmul(pc, kts[g][:, jsl], qts[g][:, isl],
                                         start=True, stop=False)
                    nc.tensor.matmul(pc, sdiag_t, rbms[t],
                                     start=False, stop=True)
                    em = pel.tile([128, 512], bf16, tag="em", name=f"em_{t}")
                    nc.scalar.activation(em, pc, AF.Exp,
                                         bias=b_exp[:128, :])
                else:
                    nc.tensor.matmul(pc, kts[g][:, jsl], qts[g][:, isl],
                                     start=True, stop=True)
                    e_in = _fallback_logits(t, pc)
                    et = pel.tile([128, 512], bf16, tag="et", name=f"et_{t}")
                    nc.scalar.activation(et, e_in, AF.Exp,
                                         bias=b_exp[:128, :])
                    em = pel.tile([128, 512], bf16, tag="em", name=f"em_{t}")
                    nc.vector.tensor_mul(em, et, rbms.pop(t))
                nc.tensor.matmul(
                    aggs[(g, ih)],
                    vexts[g][:, jc, :],
                    em,
                    start=(jc == ih_seq[ih][0]), stop=(jc == ih_seq[ih][-1]),
                )
                if jc == ih_seq[ih][-1]:
                    u_sb = pel.tile([D + 1, 512], f32, tag="usb", bufs=2,
                                    name=f"usb{g}_{ih}")
                    nc.vector.tensor_copy(u_sb, aggs[(g, ih)])
                    nc.sync.dma_start(
                        u_out[g, :, ih * 512 : (ih + 1) * 512], u_sb
                    )

            def _fallback_logits(t, pc):
                # general radial MLP: dist; sum_r w2r*relu(w1r*dist+b1r)
                lt_prev = rbs_d.pop(t)  # f32 dist
                racc = None
                for r in range(len(w1v)):
                    h = pel.tile([128, 512], f32, tag="hrad", bufs=4,
                                 name=f"h_{t}_{r}")
                    nc.scalar.activation(h, lt_prev, AF.Relu,
                                         bias=b_rad[r][:128, :],
                                         scale=float(w1v[r]))
                    nr = pel.tile([128, 512], f32, tag="racc", bufs=4,
                                  name=f"ra_{t}_{r}")
                    if racc is None:
                        nc.vector.tensor_single_scalar(
                            nr, h, float(w2v[r]), OP.mult)
                    else:
                        nc.vector.scalar_tensor_tensor(
                            nr, h, float(w2v[r]), racc, OP.mult, OP.add)
                    racc = nr
                s1 = pel.tile([128, 512], f32, tag="s1", name=f"s1_{t}")
                nc.vector.scalar_tensor_tensor(
                    s1, racc, 1.0, pc, OP.bypass, OP.add)
                return s1

            nT = len(tiles)
            half = nT // 2
            for t in range(half):
                emit_front(t)
            for i in range(half):
                emit_front(half + i)
                emit_back(i)
            for i in range(half, nT):
                emit_back(i)

    _split_matmul_waits()
    return nc


def kernel(
    feats, coors, adj_mat, Wq, Wk, Wv, Wo,
    w_r1, b_r1, w_r2, b_r2, w1, b1, w2, b2,
):
    global last_results
    f32 = np.float32
    fp8np = mybir.dt.np(mybir.dt.float8e4)

    feats = np.asarray(feats, dtype=f32)
    coors = np.asarray(coors, dtype=f32)
    adj = np.asarray(adj_mat).astype(bool)
    Wq = np.asarray(Wq, f32); Wk = np.asarray(Wk, f32)
    Wv = np.asarray(Wv, f32); Wo = np.asarray(Wo, f32)
    w_r1 = np.asarray(w_r1, f32); b_r1 = np.asarray(b_r1, f32)
    w_r2 = np.asarray(w_r2, f32); b_r2 = np.asarray(b_r2, f32)
    w1 = np.asarray(w1, f32); b1 = np.asarray(b1, f32)
    w2 = np.asarray(w2, f32); b2 = np.asarray(b2, f32)

    # ---- host layout prep (no model compute beyond O(B*N)) ----
    eye = np.eye(N, dtype=bool)
    m8 = (adj | eye[None]).astype(fp8np)  # [B,N,N] fp8 {0,1}

    fttT = np.ascontiguousarray(feats.transpose(0, 2, 1))  # [B,5,N]

    n2 = (coors * coors).sum(-1)  # [B,N]
    ones = np.ones_like(n2)
    ct = coors.transpose(0, 2, 1)  # [B,3,N]
    extL = np.concatenate([-2.0 * ct, n2[:, None], ones[:, None]], axis=1)
    extR = np.concatenate([ct, ones[:, None], n2[:, None]], axis=1)  # [B,5,N]
    lhi, llo = _bf16_split(extL)
    rhi, rlo = _bf16_split(extR)
    # sum over 20 rows = LhiRhi + LhiRlo + LloRhi + LloRlo
    extl20 = np.concatenate([lhi, lhi, llo, llo], axis=1)  # [B,20,N]
    extr20 = np.concatenate([rhi, rlo, rhi, rlo], axis=1)

    wqkv = np.concatenate([Wq / np.sqrt(D), Wk, Wv], axis=1).astype(f32)

    # radial MLP fast-path constant: rbias(d) = c*d (+const) when b_r1 == 0
    w1v = w_r1[0]  # [8]
    w2v = w_r2[:, 0]  # [8]
    fast_path = bool(np.all(b_r1 == 0.0))
    c_val = float(np.sum(np.where(w1v > 0, w1v * w2v, 0.0)))

    fp16np = mybir.dt.np(mybir.dt.float16)
    sdiag = ((BIG if fast_path else 1.0)
             * np.eye(128, dtype=np.float32)).astype(fp16np)
    ident_m = np.eye(128, dtype=np.float32).astype(fp16np)

    nc = _build(c_val, fast_path, w1v, b_r1, w2v)

    ext40 = np.concatenate([extl20, extr20], axis=2)  # [B,20,2N]
    fw = np.concatenate(
        [np.broadcast_to(wqkv[None], (B, D, 3 * D)), fttT], axis=2
    ).astype(f32)  # [B,5,15+N]

    in_maps = []
    for core in range(NCORES):
        gs = slice(core * GPC, (core + 1) * GPC)
        in_maps.append(
            {
                "m8": np.ascontiguousarray(m8[gs]),
                "ext": np.ascontiguousarray(ext40[gs]),
                "fw": np.ascontiguousarray(fw[gs]),
                "sdiag": sdiag,
                "ident": ident_m,
            }
        )

    trace = bool(os.environ.get("BASS_TRACE"))
    res = run_bass_kernel_spmd(nc, in_maps, list(range(NCORES)), trace=trace)
    last_results = res

    # ---- host finish: normalize, pool, tiny MLP head ----
    u = np.stack([r["u_out"] for r in res.results]).reshape(B, D + 1, N)
    aggT = u[:, 0:D, :] / u[:, D : D + 1, :]  # [B,5,N]
    agg_mean = aggT.mean(axis=2)  # [B,5]
    pooled = feats.mean(axis=1) + agg_mean @ Wo  # [B,5]
    hdn = np.maximum(pooled @ w1 + b1, 0.0)
    out = hdn @ w2 + b2
    return out.astype(f32)
